# revision 1
# baseline (speedup 1.0000x reference)
"""EpsSupInfoNCE loss on 8 Trainium2 NeuronCores.

Math (reference): logits = (E @ E.T)/temp;  same[i,j] = (label_i == label_j)
  S_j   = sum_i exp(logits[i,j]) * (1 - same[i,j])     (masked column sums)
  ce_ij = log(exp(-eps) + S_j * exp(-logits[i,j]))     for same-label i != j
  loss  = sum_j (1/count_j) * sum_i ce_ij / B

Strategy: columns are sharded over 8 cores; the HOST sorts all columns by
label first. Core c owns 1024 sorted columns whose same-label rows then
live in ONE contiguous row interval, padded to a 2048-row "window" (max
class size ~130, so 1024 + 2*130 always fits). Rows are passed to each
core pre-split into et_win [D,2048] and et_main [D,6144] (order of rows
is irrelevant for the sums), which keeps the program SPMD-identical
across cores while the actual window offset varies per core.

Per 128-column tile:
  main rows:   logits matmul (fp32r, single PE pass) -> PSUM;
               ACT exp(+l/temp) with fused accum_out -> S partials.
               No label mask needed: no same-label pairs here.
  window rows: logits matmul + one-hot mask matmul (-4.5 -> -C in logit
               units) -> PSUM; ACT exp(l/temp - C*same) -> Pwin, whose
               fused accum IS the masked S_win partial; DVE reciprocal
               R = 1/Pwin; ACT Ln(m_j * R + 1) with accum_out -> A_j
               partials, m_j = S_j*e^(eps-C): equals ce+eps at same-label
               entries, ~1e-17 at different-label ones.
Host: numer_sum_j = A_j - eps*count_j - log1p(S_j e^(eps-l_jj)) (the
diagonal term, which carries its own +eps), then a tiny f64 reduction. Out-of-window same-label
terms do not exist; out-of-window Ln terms are < 1e-13 and are dropped.
"""
import numpy as np
import ml_dtypes
from contextlib import ExitStack

import concourse.bacc as bacc
import concourse.tile as tile
from concourse import mybir
from concourse.bass_utils import run_bass_kernel_spmd

B, D = 8192, 128
NCLS = 100
NCORES = 8
COLS = B // NCORES            # 1024 columns per core
NCT = COLS // 128             # 8 col-tiles per core
WIN = 2048                    # window rows per core
MAIN = B - WIN                # 6144 main rows per core
GROUP = 2048                  # rows per PSUM group (4 banks)
NGM = MAIN // GROUP           # 3 main groups
NSUB = GROUP // 512
LNW = 1536                    # Ln/recip slice: true same-label block size

TEMP = 0.07
EPS = 0.25
SCALE = float(np.float32(1.0) / np.float32(TEMP))   # exp scale (fp32 value)
MASKVAL = -4.5                                      # bf16-exact additive mask
C_USED = 4.5 * SCALE                                # mask size in logit units
MCONST = float(np.exp(EPS - C_USED))                # e^(eps-C)

_cache = {}


def _patch_act_tables():
    """Steer Exp and Ln onto the one table set holding both, so Exp/Ln
    alternation doesn't thrash ACT_TABLE_LOADs. Set ids are indices into
    act_info.json, so keep dict length/order and just hide exp/ln
    elsewhere."""
    import concourse.hw_specs as hw_specs
    from concourse import mybir as _mb
    if getattr(bacc, "_act_tables_patched", False):
        return
    orig = hw_specs.get_activation_tables

    def steer(arch):
        t = orig(arch)
        exp, ln = (_mb.ActivationFunctionType.Exp, _mb.ActivationFunctionType.Ln)
        if "natural_log_exp_and_others" not in t:
            return t
        return {k: (fns if k == "natural_log_exp_and_others"
                    else fns - {exp, ln}) for k, fns in t.items()}

    bacc.get_activation_tables = steer
    bacc._act_tables_patched = True


def _build():
    dt = mybir.dt
    _patch_act_tables()
    nc = bacc.Bacc("TRN2", target_bir_lowering=False, debug=False,
                   num_devices=NCORES)
    et_main = nc.dram_tensor("et_main", [D, MAIN], dt.float32,
                             kind="ExternalInput").ap()
    et_win = nc.dram_tensor("et_win", [D, WIN], dt.float32,
                            kind="ExternalInput").ap()
    et_own = nc.dram_tensor("et_own", [D, COLS], dt.float32,
                            kind="ExternalInput").ap()
    oh_win = nc.dram_tensor("oh_win", [NCLS, WIN], dt.bfloat16,
                            kind="ExternalInput").ap()
    ohn_own = nc.dram_tensor("ohn_own", [NCLS, COLS], dt.bfloat16,
                             kind="ExternalInput").ap()
    NSLOT = NGM + 1                                    # S slots per col-tile
    out = nc.dram_tensor("out", [128, NCT * NSLOT + NCT], dt.float32,
                         kind="ExternalOutput").ap()

    with tile.TileContext(nc) as tc:
        with ExitStack() as ctx:
            const_pool = ctx.enter_context(tc.tile_pool(name="consts", bufs=1))
            p_pool = ctx.enter_context(tc.tile_pool(name="pwin", bufs=2))
            r_pool = ctx.enter_context(tc.tile_pool(name="rbuf", bufs=2))
            d_pool = ctx.enter_context(tc.tile_pool(name="dump", bufs=2))
            stage_pool = ctx.enter_context(tc.tile_pool(name="stage", bufs=2))
            ps_pool = ctx.enter_context(
                tc.tile_pool(name="psum", bufs=2, space="PSUM"))

            # fp32r matmul operands must be rounded by a compute op: DMA to
            # fp32 staging, DVE-copy (rounds) into fp32r tiles. Load order:
            # what the first col-tile needs first.
            # Load order mirrors first-consumption order: ct0's window work
            # needs et_own[:, :128], et_win, oh_win, ohn_own; the main
            # groups then need et_main chunks. Two DMA queues in parallel.
            # ct0's first matmul needs only et_own[:, :128] and et_win chunk
            # 0, so chunk those finely and alternate DMA queues to overlap
            # transfer with the first window matmuls.
            t_et_own = const_pool.tile([D, COLS], dt.float32r)
            st0 = stage_pool.tile([D, COLS], dt.float32, tag="st_own")
            nc.sync.dma_start(st0[:, 0:512], et_own[:, 0:512])
            nc.vector.tensor_copy(t_et_own[:, 0:512], st0[:, 0:512])
            t_et_win = const_pool.tile([D, WIN], dt.float32r)
            stw = stage_pool.tile([D, GROUP], dt.float32, tag="st_win")
            nc.gpsimd.dma_start(stw[:], et_win[:])
            for h in range(2):
                nc.vector.tensor_copy(t_et_win[:, h * 1024:(h + 1) * 1024],
                                      stw[:, h * 1024:(h + 1) * 1024])
            t_oh_win = const_pool.tile([NCLS, WIN], dt.bfloat16)
            nc.sync.dma_start(t_oh_win[:], oh_win[:])
            t_ohn_own = const_pool.tile([NCLS, COLS], dt.bfloat16)
            nc.sync.dma_start(t_ohn_own[:], ohn_own[:])
            nc.sync.dma_start(st0[:, 512:], et_own[:, 512:])
            nc.vector.tensor_copy(t_et_own[:, 512:], st0[:, 512:])
            t_et_main = const_pool.tile([D, MAIN], dt.float32r)
            for i in range(NGM):
                st = stage_pool.tile([D, GROUP], dt.float32, tag="st")
                nc.sync.dma_start(st[:], et_main[:, i * GROUP:(i + 1) * GROUP])
                for h in range(2):
                    lo = i * GROUP + h * 1024
                    nc.vector.tensor_copy(t_et_main[:, lo:lo + 1024],
                                          st[:, h * 1024:(h + 1) * 1024])

            s_part = const_pool.tile([128, NCT * NSLOT], dt.float32)
            a_part = const_pool.tile([128, NCT], dt.float32)
            m_raw = const_pool.tile([128, NCT], dt.float32)
            m_sb = const_pool.tile([128, NCT], dt.float32)

            def emit_ln(ct, R):
                # ce-sum: Ln(m_j / Pwin + 1), fused per-column accumulate.
                dump = d_pool.tile([128, LNW], dt.float32, tag="dump2")
                nc.scalar.activation(
                    dump[:], R[:], mybir.ActivationFunctionType.Ln,
                    scale=m_sb[:, ct:ct + 1], bias=1.0,
                    accum_out=a_part[:, ct:ct + 1])

            prev = None          # (ct, P) whose Ln is deferred one col-tile
            for ct in range(NCT):
                lhs_et = t_et_own[:, ct * 128:(ct + 1) * 128]
                lhs_oh = t_ohn_own[:, ct * 128:(ct + 1) * 128]

                # ---- window rows first: masked; feed S and the ce sum.
                # Doing these before the main groups starts the DVE S/m
                # chain early so the deferred Ln is never waiting.
                ps = ps_pool.tile([128, GROUP], dt.float32, tag="ps")
                for n in range(NSUB):
                    nc.tensor.matmul(
                        ps[:, n * 512:(n + 1) * 512], lhs_et,
                        t_et_win[:, n * 512:(n + 1) * 512],
                        start=True, stop=False)
                for n in range(NSUB):
                    nc.tensor.matmul(
                        ps[:, n * 512:(n + 1) * 512], lhs_oh,
                        t_oh_win[:, n * 512:(n + 1) * 512],
                        start=False, stop=True)
                # Pwin = exp(l - C*same); its fused accum IS the masked S_win.
                # The Ln input exp(-l + C*same) comes from the DVE reciprocal.
                P = p_pool.tile([128, WIN], dt.float32, tag="P")
                slot = ct * NSLOT + NGM
                nc.scalar.activation(
                    P[:], ps[:], mybir.ActivationFunctionType.Exp,
                    scale=SCALE, accum_out=s_part[:, slot:slot + 1])
                # Same-label rows sit at window offset 0 (host layout), so
                # the numerator path only needs the first LNW rows.
                R = r_pool.tile([128, LNW], dt.float32, tag="R")
                nc.vector.reciprocal_approx_fast(out=R[:], in_=P[:, 0:LNW])

                # ---- main rows: unmasked, only feed S ----
                for g in range(NGM):
                    r0 = g * GROUP
                    ps = ps_pool.tile([128, GROUP], dt.float32, tag="ps")
                    for n in range(NSUB):
                        nc.tensor.matmul(
                            ps[:, n * 512:(n + 1) * 512], lhs_et,
                            t_et_main[:, r0 + n * 512:r0 + (n + 1) * 512],
                            start=True, stop=True)
                    dump = d_pool.tile([128, GROUP], dt.float32, tag="dump")
                    slot = ct * NSLOT + g
                    nc.scalar.activation(
                        dump[:], ps[:], mybir.ActivationFunctionType.Exp,
                        scale=SCALE, accum_out=s_part[:, slot:slot + 1])

                nc.vector.reduce_sum(
                    m_raw[:, ct:ct + 1],
                    s_part[:, ct * NSLOT:(ct + 1) * NSLOT],
                    axis=mybir.AxisListType.X)
                nc.vector.tensor_scalar_mul(
                    m_sb[:, ct:ct + 1], m_raw[:, ct:ct + 1], MCONST)

                # Defer this tile's Ln so the ACT FIFO can run the next
                # tile's exps while the DVE S/m chain completes.
                if prev is not None:
                    emit_ln(*prev)
                prev = (ct, R)
            emit_ln(*prev)

            nc.sync.dma_start(out[:, 0:NCT * NSLOT], s_part[:])
            nc.sync.dma_start(out[:, NCT * NSLOT:], a_part[:])
    nc.compile()
    return nc


def _get_nc():
    if "nc" not in _cache:
        _cache["nc"] = _build()
    return _cache["nc"]


def _prepare(embeds, labels):
    embeds = np.ascontiguousarray(np.asarray(embeds, dtype=np.float32))
    labels_i = np.asarray(labels).astype(np.int64)
    assert embeds.shape == (B, D)

    # Sort columns (and rows -- it is the same axis) by label so each
    # core's same-label rows are contiguous.
    perm = np.argsort(labels_i, kind="stable")
    lab = labels_i[perm]
    emb = embeds[perm]

    et = np.ascontiguousarray(emb.T)                      # [D, B] f32
    oh = np.zeros((NCLS, B), dtype=ml_dtypes.bfloat16)
    oh[lab, np.arange(B)] = ml_dtypes.bfloat16(1.0)
    ohn = (oh.astype(np.float32) * np.float32(MASKVAL)).astype(ml_dtypes.bfloat16)

    # class start/end in sorted order
    starts = np.searchsorted(lab, np.arange(NCLS), side="left")
    ends = np.searchsorted(lab, np.arange(NCLS), side="right")

    in_maps = []

    for c in range(NCORES):
        lo, hi = c * COLS, (c + 1) * COLS
        r_lo = int(starts[lab[lo]])
        r_hi = int(ends[lab[hi - 1]])
        span = r_hi - r_lo
        assert span <= LNW, f"window overflow: {span}"
        fill = WIN - span
        after = np.arange(r_hi, min(B, r_hi + fill))
        need = fill - len(after)
        before = np.arange(r_lo - need, r_lo) if need > 0 else np.arange(0)
        win_rows = np.concatenate([np.arange(r_lo, r_hi), after, before])
        assert len(win_rows) == WIN
        main_mask = np.ones(B, dtype=bool)
        main_mask[win_rows] = False
        main_idx = np.nonzero(main_mask)[0]
        in_maps.append({
            "et_main": np.ascontiguousarray(et[:, main_idx]),
            "et_win": np.ascontiguousarray(et[:, win_rows]),
            "et_own": np.ascontiguousarray(et[:, lo:hi]),
            "oh_win": np.ascontiguousarray(oh[:, win_rows]),
            "ohn_own": np.ascontiguousarray(ohn[:, lo:hi]),
        })
    return in_maps, lab, emb


def _combine(results, lab, emb):
    NSLOT = NGM + 1
    S = np.empty(B, dtype=np.float64)
    A = np.empty(B, dtype=np.float64)
    for c in range(NCORES):
        o = results[c]["out"].astype(np.float64)
        s = o[:, :NCT * NSLOT].reshape(128, NCT, NSLOT).sum(-1)   # [p, ct]
        a = o[:, NCT * NSLOT:NCT * NSLOT + NCT]                   # [p, ct]
        S[c * COLS:(c + 1) * COLS] = s.T.reshape(-1)              # j = ct*128+p
        A[c * COLS:(c + 1) * COLS] = a.T.reshape(-1)

    counts = np.bincount(lab, minlength=NCLS)
    count_j = counts[lab].astype(np.float64) - 1.0
    l_jj = (emb.astype(np.float64) ** 2).sum(1) * SCALE
    # A_j = sum_{in_numer}(ce+eps) + (ce_jj+eps); u_jj = ce_jj + eps, so
    # numer = A_j - eps*count_j - u_jj.
    u_jj = np.log1p(S * np.exp(EPS - l_jj))
    numer = A - EPS * count_j - u_jj
    loss = (numer / count_j).sum() / B
    return np.asarray(loss, dtype=np.float32)


def kernel(embeds, labels):
    in_maps, lab, emb = _prepare(embeds, labels)
    nc = _get_nc()
    res = run_bass_kernel_spmd(nc, in_maps, list(range(NCORES)))
    return _combine(res.results, lab, emb)



# revision 10
# speedup vs baseline: 1.0291x; 1.0291x over previous
"""EpsSupInfoNCE loss on 8 Trainium2 NeuronCores — v2.

Math (reference): logits = (E @ E.T)/temp;  same[i,j] = (label_i == label_j)
  S_j   = sum_i exp(logits[i,j]) * (1 - same[i,j])     (masked column sums)
  ce_ij = log(exp(-eps) + S_j * exp(-logits[i,j]))     for same-label i != j
  loss  = sum_j (1/count_j) * sum_i ce_ij / B

Columns are sharded over 8 cores after a host-side label sort, so core c's
1024 columns have all their same-label rows in one contiguous "window" of
W=1280 sorted rows.  Everything runs in bf16 (tolerance is 2e-2; bf16
perturbs logits/temp by ~4e-3).

Per core the 8192 rows split three ways, each row counted exactly once:
  window rows (1280, column layout, PSUM X): ACT exp with fused column
    accum, skipping the per-tile 448-row label slice; the one-hot mask
    matmul accumulates -4.5/temp into the slice range of X so the slice
    activation sees masked logits (same-label entries vanish from S).
  mainB rows (1280, column layout, PSUM Y): ACT exp + fused accum.
  trans rows (5632, 44 transposed 128-row blocks, PSUM T0/T1): bf16 matmul
    -> fp32 PSUM -> DVE tensor_scalar affine to int16 whose bit pattern IS
    bf16 exp (Schraudolph), -> PE ones-matmul sums over rows into four
    [1,256] PSUM quarter-accumulators.  A few blocks run their exp on ACT
    (exact, bf16 out) to balance the engines.
Numerator per tile: slice logits (masked) -> ACT exp -> P (+ accum a1 =
slice's different-label S part) -> DVE reciprocal -> ACT Ln(m_j/P + 1) with
fused accum = sum of (ce+eps) over the tile's same-label rows; m_j =
S_j*e^(eps-C).  Host subtracts eps*count and the diagonal term.
"""
import numpy as np
import ml_dtypes
from contextlib import ExitStack

import concourse.bacc as bacc
import concourse.tile as tile
from concourse import mybir
from concourse.bass_utils import run_bass_kernel_spmd

B, D = 8192, 128
NCLS = 100
NCORES = 8
COLS = B // NCORES            # 1024 columns per core
NCT = COLS // 128             # 8 col-tiles per core
W = 1280                      # window rows (span + pad)
SW = 448                      # per-tile numerator slice width
YROWS = 1024                  # column-layout main rows
TR = B - W - YROWS            # 5888 transposed rows
NBLK = TR // 128              # 46 transposed blocks
NSLOT = 5                     # s_part slots/tile: X1, X2, Y, a1, scat
ACT_BLK_EVERY = 6             # every 6th trans block exps on ACT not DVE

TEMP = 0.07
EPS = 0.25
SCALE = float(np.float32(1.0) / np.float32(TEMP))
MASKVAL = -4.5                                      # bf16-exact additive mask
C_USED = 4.5 * SCALE                                # mask size in logit units
MCONST = float(np.exp(EPS - C_USED))                # e^(eps-C)
LOG2E = 1.4426950408889634
K2 = 128.0 * LOG2E * SCALE                          # Schraudolph int16/bf16
B2 = 128.0 * 127.0 - 7.42

# PSUM word layout (4096 fp32 per partition, bank = 512).  The S quarter
# accumulators live ALONE in bank 5: a start=True matmul clears has_written
# for its ENTIRE bank, which would reset any accumulation sharing the bank.
PX = 0                        # X: window logits         [0, 1280)
PY = 1536                     # Y: mainB logits          [1536, 2560)
PSA = 2560                    # S quarters 0/1           [2560, 2816)
PSB = 2816                    # S quarters 2/3           [2816, 3072)
PT0 = 3072                    # trans chunk buf 0        [3072, 3584)
PT1 = 3584                    # trans chunk buf 1        [3584, 4096)

_cache = {}


def _patch_act_tables():
    """Steer Exp and Ln onto the one table set holding both, so Exp/Ln
    alternation doesn't thrash ACT_TABLE_LOADs."""
    import concourse.hw_specs as hw_specs
    from concourse import mybir as _mb
    if getattr(bacc, "_act_tables_patched", False):
        return
    orig = hw_specs.get_activation_tables

    def steer(arch):
        t = orig(arch)
        exp, ln = (_mb.ActivationFunctionType.Exp, _mb.ActivationFunctionType.Ln)
        if "natural_log_exp_and_others" not in t:
            return t
        return {k: (fns if k == "natural_log_exp_and_others"
                    else fns - {exp, ln}) for k, fns in t.items()}

    bacc.get_activation_tables = steer
    bacc._act_tables_patched = True


def _bank_split(r0, r1):
    """Split [r0, r1) at 512-word PSUM bank boundaries (matmul outs must not
    cross banks)."""
    out = []
    while r0 < r1:
        nxt = min(r1, (r0 // 512 + 1) * 512)
        out.append((r0, nxt))
        r0 = nxt
    return out


def _build(los):
    dt = mybir.dt
    _patch_act_tables()
    nc = bacc.Bacc("TRN2", target_bir_lowering=False, debug=False,
                   num_devices=NCORES)
    et_win = nc.dram_tensor("et_win", [D, W], dt.bfloat16,
                            kind="ExternalInput").ap()
    et_y = nc.dram_tensor("et_y", [D, YROWS], dt.bfloat16,
                          kind="ExternalInput").ap()
    et_tr = nc.dram_tensor("et_tr", [D, TR], dt.bfloat16,
                           kind="ExternalInput").ap()
    et_own = nc.dram_tensor("et_own", [D, COLS], dt.bfloat16,
                            kind="ExternalInput").ap()
    oh_win = nc.dram_tensor("oh_win", [NCLS, W], dt.bfloat16,
                            kind="ExternalInput").ap()
    ohn_own = nc.dram_tensor("ohn_own", [NCLS, COLS], dt.bfloat16,
                             kind="ExternalInput").ap()
    out = nc.dram_tensor("out", [128, 2 * NCT], dt.float32,
                         kind="ExternalOutput").ap()
    out_dbg = nc.dram_tensor("out_dbg", [128, NCT * NSLOT + 512], dt.float32,
                             kind="ExternalOutput").ap()
    scratch = nc.dram_tensor("scratch", [1, COLS], dt.float32,
                             kind="Internal").ap()

    with tile.TileContext(nc) as tc:
        with ExitStack() as ctx:
            cpool = ctx.enter_context(tc.tile_pool(name="consts", bufs=1))
            dpool = ctx.enter_context(tc.tile_pool(name="dumps", bufs=2))
            ppool = ctx.enter_context(tc.tile_pool(name="pbuf", bufs=2))
            vpool = ctx.enter_context(tc.tile_pool(name="conv", bufs=2))
            ps_pool = ctx.enter_context(
                tc.tile_pool(name="psum", bufs=1, space="PSUM"))

            t_win = cpool.tile([D, W], dt.bfloat16)
            t_y = cpool.tile([D, YROWS], dt.bfloat16)
            t_tr = cpool.tile([D, TR], dt.bfloat16)
            t_own = cpool.tile([D, COLS], dt.bfloat16)
            t_oh = cpool.tile([NCLS, W], dt.bfloat16)
            t_ohn = cpool.tile([NCLS, COLS], dt.bfloat16)
            nc.sync.dma_start(t_own[:], et_own[:])
            nc.sync.dma_start(t_win[:], et_win[:])
            nc.gpsimd.dma_start(t_ohn[:], ohn_own[:])
            nc.gpsimd.dma_start(t_oh[:], oh_win[:])
            nc.sync.dma_start(t_y[:], et_y[:])
            for q in range(4):
                nc.sync.dma_start(t_tr[:, q * (TR // 4):(q + 1) * (TR // 4)],
                                  et_tr[:, q * (TR // 4):(q + 1) * (TR // 4)])

            ones = cpool.tile([128, 1], dt.bfloat16)
            nc.vector.memset(ones[:], 1.0)
            s_part = cpool.tile([128, NCT * NSLOT], dt.float32)
            nc.vector.memset(s_part[:], 0.0)
            a_part = cpool.tile([128, NCT], dt.float32)
            s_sb = cpool.tile([128, NCT], dt.float32)
            m_sb = cpool.tile([128, NCT], dt.float32)
            rbig = cpool.tile([128, NCT, SW], dt.float32)
            ssb = cpool.tile([128, 2, 256], dt.float32)

            big = ps_pool.tile([128, 4096], dt.float32)

            def quarter(q):
                base = PSA if q < 2 else PSB
                p = 32 * (q % 2)
                return big[p:p + 1, base:base + 256]

            def trans_block(blk):
                lhs = t_tr[:, blk * 128:(blk + 1) * 128]
                on_act = (blk % ACT_BLK_EVERY) == (ACT_BLK_EVERY - 1)
                for h, pt in ((0, PT0), (1, PT1)):
                    tbuf = big[:, pt:pt + 512]
                    nc.tensor.matmul(tbuf, lhs,
                                     t_own[:, h * 512:(h + 1) * 512],
                                     start=True, stop=True,
                                     skip_group_check=True)
                    if on_act:
                        cb = vpool.tile([128, 512], dt.bfloat16,
                                        tag=f"ab{h}")
                        nc.scalar.activation(
                            cb[:], tbuf, mybir.ActivationFunctionType.Exp,
                            scale=SCALE)
                        subs = [cb[:, 0:256], cb[:, 256:512]]
                    else:
                        cb = vpool.tile([128, 512], dt.int16, tag=f"cv{h}")
                        nc.vector.tensor_scalar(
                            out=cb[:], in0=tbuf, scalar1=K2, scalar2=B2,
                            op0=mybir.AluOpType.mult,
                            op1=mybir.AluOpType.add)
                        subs = [cb[:, 0:256].bitcast(dt.bfloat16),
                                cb[:, 256:512].bitcast(dt.bfloat16)]
                    for sub in range(2):
                        nc.tensor.matmul(
                            quarter(2 * h + sub), ones[:], subs[sub],
                            start=(blk == 0), stop=(blk == NBLK - 1),
                            skip_group_check=True)

            blk_iter = iter(range(NBLK))

            for ct in range(NCT):
                lo = los[ct]
                lhs_et = t_own[:, ct * 128:(ct + 1) * 128]
                lhs_oh = t_ohn[:, ct * 128:(ct + 1) * 128]

                # ---- X: window logits ----
                for r0, r1 in ((0, 512), (512, 1024), (1024, W)):
                    nc.tensor.matmul(big[:, PX + r0:PX + r1], lhs_et,
                                     t_win[:, r0:r1], start=True, stop=False,
                                     skip_group_check=True)
                # mask the slice range (adds -4.5*same in logit units)
                for r0, r1 in _bank_split(lo, lo + SW):
                    nc.tensor.matmul(big[:, PX + r0:PX + r1], lhs_oh,
                                     t_oh[:, r0:r1], start=False, stop=True,
                                     skip_group_check=True)

                # window exp, skipping the slice; fused column partial sums
                xd = dpool.tile([128, W], dt.bfloat16, tag="xdump")
                if lo > 0:
                    nc.scalar.activation(
                        xd[:, 0:lo], big[:, PX:PX + lo],
                        mybir.ActivationFunctionType.Exp, scale=SCALE,
                        accum_out=s_part[:, ct * NSLOT:ct * NSLOT + 1])
                nc.scalar.activation(
                    xd[:, lo + SW:W], big[:, PX + lo + SW:PX + W],
                    mybir.ActivationFunctionType.Exp, scale=SCALE,
                    accum_out=s_part[:, ct * NSLOT + 1:ct * NSLOT + 2])

                # slice: masked exp -> P (numerator) + a1 (different-label sum)
                P = ppool.tile([128, SW], dt.float32, tag="P")
                nc.scalar.activation(
                    P[:], big[:, PX + lo:PX + lo + SW],
                    mybir.ActivationFunctionType.Exp, scale=SCALE,
                    accum_out=s_part[:, ct * NSLOT + 3:ct * NSLOT + 4])
                nc.vector.reciprocal_approx_fast(
                    out=rbig[:, ct, :], in_=P[:])

                # ---- Y: mainB logits ----
                for r0, r1 in ((0, 512), (512, YROWS)):
                    nc.tensor.matmul(big[:, PY + r0:PY + r1], lhs_et,
                                     t_y[:, r0:r1], start=True, stop=True,
                                     skip_group_check=True)
                yd = dpool.tile([128, YROWS], dt.bfloat16, tag="ydump")
                nc.scalar.activation(
                    yd[:], big[:, PY:PY + YROWS],
                    mybir.ActivationFunctionType.Exp, scale=SCALE,
                    accum_out=s_part[:, ct * NSLOT + 2:ct * NSLOT + 3])

                # ---- interleave transposed blocks ----
                nblk_here = 6 if ct < 7 else 4
                for _ in range(nblk_here):
                    trans_block(next(blk_iter))

            # ---- S quarters -> scratch DRAM -> per-tile slot scatter ----
            nc.vector.tensor_copy(ssb[:, 0, :], big[:, PSA:PSA + 256])
            nc.vector.tensor_copy(ssb[:, 1, :], big[:, PSB:PSB + 256])
            # quarter q=2*reg+a lives at ssb[32a, reg, :]; col j = q*256+r
            nc.sync.dma_start(
                scratch[:].rearrange("o (reg ph r) -> ph o reg r",
                                     ph=2, reg=2),
                ssb[:].rearrange("(a b) reg r -> a b reg r", b=32)[0:2, 0:1])
            nc.sync.dma_start(
                s_part[:].rearrange("p (c s) -> p c s", s=NSLOT)[:, :, 4:5],
                scratch[:].rearrange("o (c p) -> (o p) c", p=128))

            # ---- m_j and the deferred Ln's ----
            for ct in range(NCT):
                nc.vector.reduce_sum(
                    s_sb[:, ct:ct + 1],
                    s_part[:, ct * NSLOT:(ct + 1) * NSLOT],
                    axis=mybir.AxisListType.X)
                nc.vector.tensor_scalar_mul(
                    m_sb[:, ct:ct + 1], s_sb[:, ct:ct + 1], MCONST)
                ld = dpool.tile([128, SW], dt.bfloat16, tag="lndump")
                nc.scalar.activation(
                    ld[:], rbig[:, ct, :], mybir.ActivationFunctionType.Ln,
                    scale=m_sb[:, ct:ct + 1], bias=1.0,
                    accum_out=a_part[:, ct:ct + 1])

            nc.sync.dma_start(out[:, 0:NCT], a_part[:])
            nc.sync.dma_start(out[:, NCT:], s_sb[:])
            nc.sync.dma_start(out_dbg[:, 0:NCT * NSLOT], s_part[:])
            nc.sync.dma_start(
                out_dbg[:, NCT * NSLOT:].rearrange(
                    "p (reg r) -> p reg r", reg=2), ssb[:])
    nc.compile()
    return nc


def _get_nc(los):
    key = ("v2", tuple(los))
    if key not in _cache:
        _cache[key] = _build(tuple(los))
    return _cache[key]


def _prepare(embeds, labels):
    embeds = np.ascontiguousarray(np.asarray(embeds, dtype=np.float32))
    labels_i = np.asarray(labels).astype(np.int64)
    assert embeds.shape == (B, D)

    perm = np.argsort(labels_i, kind="stable")
    lab = labels_i[perm]
    emb = embeds[perm]
    ebf = emb.astype(ml_dtypes.bfloat16)
    et = np.ascontiguousarray(ebf.T)                      # [D, B] bf16
    oh = np.zeros((NCLS, B), dtype=ml_dtypes.bfloat16)
    oh[lab, np.arange(B)] = ml_dtypes.bfloat16(1.0)
    ohn = oh * ml_dtypes.bfloat16(MASKVAL)

    starts = np.searchsorted(lab, np.arange(NCLS), side="left")
    ends = np.searchsorted(lab, np.arange(NCLS), side="right")

    # per-tile slice offsets (shared across cores: SPMD)
    s_min = [10**9] * NCT
    e_max = [0] * NCT
    spans = []
    for c in range(NCORES):
        lo, hi = c * COLS, (c + 1) * COLS
        r_lo = int(starts[lab[lo]])
        r_hi = int(ends[lab[hi - 1]])
        spans.append((r_lo, r_hi))
        assert r_hi - r_lo <= W, f"span overflow: {r_hi - r_lo}"
        for ct in range(NCT):
            cl, ch = lo + ct * 128, lo + (ct + 1) * 128
            s_ct = int(starts[lab[cl]]) - r_lo
            e_ct = int(ends[lab[ch - 1]]) - r_lo
            s_min[ct] = min(s_min[ct], s_ct)
            e_max[ct] = max(e_max[ct], e_ct)
    los = []
    for ct in range(NCT):
        lo_ct = max(0, min(e_max[ct] - SW, s_min[ct], W - SW))
        assert lo_ct <= s_min[ct] and e_max[ct] <= lo_ct + SW, (
            f"slice infeasible ct={ct}: [{s_min[ct]},{e_max[ct]}] "
            f"vs lo={lo_ct} SW={SW}")
        los.append(lo_ct)

    in_maps = []
    for c in range(NCORES):
        r_lo, r_hi = spans[c]
        span = r_hi - r_lo
        fill = W - span
        after = np.arange(r_hi, min(B, r_hi + fill))
        need = fill - len(after)
        before = np.arange(r_lo - need, r_lo) if need > 0 else np.arange(0)
        win_rows = np.concatenate([np.arange(r_lo, r_hi), after, before])
        assert len(win_rows) == W
        main_mask = np.ones(B, dtype=bool)
        main_mask[win_rows] = False
        main_idx = np.nonzero(main_mask)[0]
        assert len(main_idx) == YROWS + TR
        lo = c * COLS
        in_maps.append({
            "et_win": np.ascontiguousarray(et[:, win_rows]),
            "et_y": np.ascontiguousarray(et[:, main_idx[:YROWS]]),
            "et_tr": np.ascontiguousarray(et[:, main_idx[YROWS:]]),
            "et_own": np.ascontiguousarray(et[:, lo:lo + COLS]),
            "oh_win": np.ascontiguousarray(oh[:, win_rows]),
            "ohn_own": np.ascontiguousarray(ohn[:, lo:lo + COLS]),
        })
    return in_maps, los, lab, ebf


def _combine(results, lab, ebf):
    S = np.empty(B, dtype=np.float64)
    A = np.empty(B, dtype=np.float64)
    for c in range(NCORES):
        o = results[c]["out"].astype(np.float64)
        A[c * COLS:(c + 1) * COLS] = o[:, 0:NCT].T.reshape(-1)
        S[c * COLS:(c + 1) * COLS] = o[:, NCT:].T.reshape(-1)

    counts = np.bincount(lab, minlength=NCLS)
    count_j = counts[lab].astype(np.float64) - 1.0
    l_jj = (ebf.astype(np.float64) ** 2).sum(1) * SCALE
    u_jj = np.log1p(S * np.exp(EPS - l_jj))
    numer = A - EPS * count_j - u_jj
    loss = (numer / count_j).sum() / B
    return np.asarray(loss, dtype=np.float32)


def kernel(embeds, labels):
    in_maps, los, lab, ebf = _prepare(embeds, labels)
    nc = _get_nc(los)
    res = run_bass_kernel_spmd(nc, in_maps, list(range(NCORES)))
    return _combine(res.results, lab, ebf)
